# revision 4
# baseline (speedup 1.0000x reference)
"""Trainium2 Bass kernel for nn_ComputeEdgeLoss — v2.

Computes, for each batch b and lower-triangular pair (i, j) of the 64
recon keypoints, the mean over 5 interpolated segment points of the min
squared distance to the 2048 gt points of that batch.

Strategy (v2 changes over the 79 us baseline)
---------------------------------------------
* 24 row-tiles instead of 25: the 64 endpoint rows are split 32/32
  between the two cores of each batch pair (host assembly re-merges),
  so each core computes 3*1008 + 32 = 3056 rows in 24 128-row tiles.
  PE work drops 4%.
* Input columns reordered [PF_t0 | GT | PF_rest] and split into 5 DMAs
  so the first matmul's dependencies (PF tile 0 + GT chunks 0-1) arrive
  in one small early DMA instead of after the full 420 KB load.
* Tail: the last two tiles are DVE-direct and the final ACT fold batch
  lands at tile 21, so only one ~1.5 us reduce trails the final matmul
  vs ~7 us of serial DVE work before; ACT-route results ship in an
  early DMA so the final transfer is 12 columns.

Math: for an interp point k and gt point g,
    ||k - g||^2 = a . b,  a = [kx, ky, kz, ||k||^2, 1],
                          b = [-2gx, -2gy, -2gz, 1, ||g||^2]
fp32 inputs split into three bf16 terms (27-bit fidelity) and the six
>=2^-24 product groups folded into 40 bf16 contraction rows; one PE
matmul emits a [128 x 512] fp32 distance block per 512 cycles.
"""

import numpy as np

import concourse.bass as bass
import concourse.mybir as mybir
import concourse.tile as tile
from concourse.bass_utils import run_bass_kernel_spmd

# Problem shape (hardcoded per contest rules).
B = 4          # batches
NPTS = 64      # recon points per batch
M = 2048       # gt points per batch
P = NPTS * (NPTS - 1) // 2   # 2016 pairs
HALF = P // 2                # 1008 pairs per core
N_CORES = 8
FRACS = (0.25, 0.5, 0.75)    # interior interpolation fractions
NF = len(FRACS)
EPC = NPTS // 2              # endpoints per core (32)
ROWS = NF * HALF + EPC       # 3056 real rows per core
NTILES = 24                  # ceil(3056 / 128)
RPAD = NTILES * 128          # 3072 padded rows
KEXT = 40                    # split contraction depth (8 groups x 5)
GT_CHUNK = 512               # PSUM bank free size (fp32)
HM = M // 2                  # half-tile free size [128, 1024]

# AB column layout: [PF tile0 (128) | GT (2048) | PF tiles 1..23 (2944)]
AB_COLS = RPAD + M
GT0 = 128                    # GT column offset

_II, _JJ = np.tril_indices(NPTS, -1)   # pair order matches reference

# Tiles drained by DVE directly from PSUM (fp32 tensor_reduce per half).
# The rest: ACT fp16-copy to SBUF, then a batched DVE fold tree (fp16
# tensor_tensor at 2 elem/cycle) per KB tiles.  The last fold batch ends
# at tile 19 so its ~4 us chain overlaps tiles 20-23's matmuls, and the
# final four tiles are DVE-direct so only ~2 half-reduces trail the last
# matmul (a trailing fold batch cost ~5.8 us of serial DVE before).
# (Fused tensor_tensor_reduce and custom DVE ops both fail this walrus
# build's codegen with "ISA wrong length"; GpSimd has no TensorTensor.)
DVE_SET = (2, 10, 20, 21, 22, 23)
KB = 3                       # ACT tiles per batched fold chain
N_EARLY_DVE = 2              # leading DVE_SET tiles shipped in the early DMA


def _split3_bf16(x: np.ndarray):
    """Split fp32 x into three bf16 terms with x ~= h + l + r (27-bit
    significand fidelity; differences are Sterbenz-exact in fp32)."""
    import ml_dtypes

    bf16 = ml_dtypes.bfloat16
    x = np.ascontiguousarray(x, dtype=np.float32)
    h = x.astype(bf16)
    l32 = (x - h.astype(np.float32)).astype(np.float32)
    l = l32.astype(bf16)
    r = (l32 - l.astype(np.float32)).astype(np.float32).astype(bf16)
    return h, l, r


_COMPUTE_ENGINES = {"PE", "DVE", "Activation", "Pool"}


def _register_min_reduce_op():
    """Register a custom DVE op: out = min(in0, in1) elementwise, with a
    fused min-accumulate over the free dim into accum_out (init = s0).

    Stock tensor_reduce runs at 1 element/cycle on this HW, and walrus
    rejects the stock InstTensorTensorReduce ("ISA wrong length"), but a
    custom two-source DVE body consumes 2 elements/cycle: reading one
    half of each distance tile from PSUM (in0) and the ACT-staged fp16
    other half from SBUF (in1) drains a whole [128, 2048] tile in one
    ~1.3 us op."""
    import concourse.dve_ops as dops
    from concourse.dve_spec import C0, Spec, Src0, Src1, lower, minn
    from concourse.dve_uop import DveOpSpec

    name = "ANT_TT_MIN_REDUCE_EDGE"
    for o in dops.OPS:
        if o.name == name:
            return o

    def _ref(in0, in1, c0, c1, c2):
        return np.minimum(in0.astype(np.float32), in1.astype(np.float32))

    spec = Spec(body=minn(Src0, Src1), accum=minn, accum_init=C0, reference=_ref)
    row = max(dops._SUB_OPCODE_FOR_NAME.values()) + 1
    assert row < 0x20
    ver = "v3"  # TRN2
    sha = DveOpSpec(
        name=name, opcode=row, uops=lower(spec, ver=ver), rd1_en=True
    ).sha(ver)
    op = dops.DveOp(name, spec, subdim=False, uops_sha={ver: sha})
    dops.OPS.append(op)
    dops.CUSTOM_DVE_SPECS[name] = spec
    dops._SUB_OPCODE_FOR_NAME[name] = row
    return op


def _prune_redundant_waits(bir: dict) -> dict:
    """Reduce every instruction to at most ONE sync-wait.

    This walrus build accepts only one sync-wait per instruction, but
    Tile's semaphore pass is not transitively minimal.  We reconstruct
    per-instruction guaranteed semaphore lower bounds (vector clocks
    over the scheduled program order) and delete implied waits; any
    residual multi-wait instruction is split into single-wait Drain
    carriers on the same engine.

    Soundness model: per-engine in-order dispatch; in-order completion
    for compute engines; per-semaphore in-order completion for DMA-queue
    sems (each DMAHW sem belongs to one queue).  Only monotone
    (inc-only) semaphores with sem-ge-imm waits participate.
    """
    fn = bir["functions"][0]

    contrib_engines: dict[int, set] = {}
    monotone: dict[int, bool] = {}
    for b in fn["blocks"]:
        for ins in b["instructions"]:
            sy = ins.get("sync_info") or {}
            for u in sy.get("on_update") or []:
                if u.get("sync_type") != "semaphore":
                    continue
                s = u["id"]
                contrib_engines.setdefault(s, set()).add(ins.get("engine"))
                ok = u.get("update_mode") == "sem-inc"
                monotone[s] = monotone.get(s, True) and ok

    def usable(s):
        return monotone.get(s, False)

    def mergemax(dst, src):
        for k, v in src.items():
            if dst.get(k, -1) < v:
                dst[k] = v

    prev_start_know: dict[str, dict] = {}
    cum: dict[int, int] = {}            # sem -> cumulative inc in walk order
    comp_know: list[dict] = []          # per walk index
    sem_reach: dict[int, list] = {}     # sem -> [(value_after, walk_idx)]
    dropped = 0
    walk_idx = 0

    for b in fn["blocks"]:
        new_insts = []
        for ins in b["instructions"]:
            eng = ins.get("engine")
            sy = ins.get("sync_info") or {}
            waits = list(sy.get("on_wait") or [])

            def know_from(wlist):
                know = dict(prev_start_know.get(eng, {}))
                for w in wlist:
                    if (w.get("sync_type") != "semaphore"
                            or w.get("wait_mode") != "sem-ge-imm"):
                        continue
                    s, v = w["id"], w["wait_value"]
                    if not usable(s):
                        continue
                    if know.get(s, -1) < v:
                        know[s] = v
                    if len(contrib_engines.get(s, ())) == 1:
                        for after, pidx in sem_reach.get(s, ()):
                            if after >= v:
                                mergemax(know, comp_know[pidx])
                                break
                return know

            if len(waits) > 1:
                kept = list(waits)
                changed = True
                while changed and len(kept) > 1:
                    changed = False
                    for w in list(kept):
                        others = [x for x in kept if x is not w]
                        if (w.get("sync_type") == "semaphore"
                                and w.get("wait_mode") == "sem-ge-imm"
                                and usable(w["id"])
                                and know_from(others).get(w["id"], -1)
                                >= w["wait_value"]):
                            kept.remove(w)
                            dropped += 1
                            changed = True
                            break
                if len(kept) > 1:
                    for k, w in enumerate(kept[:-1]):
                        new_insts.append({
                            "name": f"{ins['name']}-w{k}",
                            "engine": eng, "ins": [], "outs": [],
                            "opcode": "Drain",
                            "sync_info": {"on_wait": [w], "on_update": []},
                        })
                        walk_idx += 1
                        comp_know.append(dict(prev_start_know.get(eng, {})))
                    kept = kept[-1:]
                if len(kept) != len(waits):
                    if not sy:
                        ins["sync_info"] = sy = {"on_update": []}
                    sy["on_wait"] = kept
                    waits = kept

            start_know = know_from(waits)
            prev_start_know[eng] = start_know

            own = set()
            for u in sy.get("on_update") or []:
                if (u.get("sync_type") == "semaphore"
                        and u.get("update_mode") == "sem-inc"):
                    s = u["id"]
                    cum[s] = cum.get(s, 0) + u.get("update_value", 1)
                    sem_reach.setdefault(s, []).append((cum[s], walk_idx))
                    own.add(s)
            ck = dict(start_know)
            for s in own:
                if usable(s) and len(contrib_engines.get(s, ())) == 1:
                    if ck.get(s, -1) < cum[s]:
                        ck[s] = cum[s]
            if eng in _COMPUTE_ENGINES:
                for s, c in cum.items():
                    if (usable(s) and contrib_engines.get(s) == {eng}
                            and ck.get(s, -1) < c):
                        ck[s] = c
            comp_know.append(ck)
            new_insts.append(ins)
            walk_idx += 1
        b["instructions"] = new_insts
    return bir


def _build_nc() -> bass.Bass:
    nc = bass.Bass()
    ab = nc.declare_dram_parameter("ab", [KEXT, AB_COLS], mybir.dt.bfloat16,
                                   isOutput=False)
    n_dve = len(DVE_SET)
    n_act = NTILES - n_dve
    assert n_act % KB == 0
    # res columns: [0:n_act] fp32 B-mins by act-ordinal; then 2 per DVE tile
    res = nc.declare_dram_parameter("res", [128, n_act + 2 * n_dve],
                                    mybir.dt.float32, isOutput=True)

    f32 = mybir.dt.float32
    f16 = mybir.dt.float16

    with tile.TileContext(nc) as tc:
        with (
            tc.tile_pool(name="const", bufs=1) as const_pool,
            tc.tile_pool(name="psum", bufs=4, space="PSUM") as psum_pool,
            tc.tile_pool(name="cp", bufs=2) as cp_pool,
            tc.tile_pool(name="fold", bufs=2) as fold_pool,
        ):
            AB = const_pool.tile([KEXT, AB_COLS], mybir.dt.bfloat16, name="AB")
            DMINS = const_pool.tile([128, 2 * n_dve], f32, name="DMINS")
            BMINS = const_pool.tile([128, n_act], f32, name="BMINS")

            # 5 input DMAs ordered by first consumer:
            #   D0 = PF_t0 + GT chunks 0-1, D1 = GT chunks 2-3,
            #   D2 = PF tiles 1-4, D3/D4 = the rest.
            cuts = (0, GT0 + 2 * GT_CHUNK, GT0 + M, GT0 + M + 4 * 128,
                    GT0 + M + 1664, AB_COLS)
            for i in range(len(cuts) - 1):
                nc.sync.dma_start(out=AB[:, cuts[i]:cuts[i + 1]],
                                  in_=ab[:, cuts[i]:cuts[i + 1]])
            GT = AB[:, GT0:GT0 + M]

            def pf_tile(t):
                if t == 0:
                    return AB[:, 0:128]
                return AB[:, GT0 + M + (t - 1) * 128:GT0 + M + t * 128]

            d_idx = 0
            a_idx = 0
            cp_cur = None
            for t in range(NTILES):
                lhsT = pf_tile(t)
                halves = []
                for hh in range(2):
                    ptile = psum_pool.tile([128, HM], f32, tag="ptile")
                    halves.append(ptile)
                    for c in range(2):
                        sl_g = slice((2 * hh + c) * GT_CHUNK,
                                     (2 * hh + c + 1) * GT_CHUNK)
                        sl_p = slice(c * GT_CHUNK, (c + 1) * GT_CHUNK)
                        nc.tensor.matmul(
                            out=ptile[:, sl_p], lhsT=lhsT, rhs=GT[:, sl_g],
                            start=True, stop=True,
                        )
                if t in DVE_SET:
                    for hh in range(2):
                        nc.vector.tensor_reduce(
                            out=DMINS[:, 2 * d_idx + hh:2 * d_idx + hh + 1],
                            in_=halves[hh][:, :],
                            axis=mybir.AxisListType.X, op=mybir.AluOpType.min,
                        )
                    d_idx += 1
                    continue

                j = a_idx % KB
                if j == 0:
                    cp_cur = cp_pool.tile([128, KB * M], f16, tag="cp")
                for hh in range(2):
                    nc.scalar.copy(
                        cp_cur[:, j * M + hh * HM:j * M + (hh + 1) * HM],
                        halves[hh][:, :])
                a_idx += 1
                if j == KB - 1:
                    b0 = a_idx - KB
                    c3 = cp_cur[:, :].rearrange("p (k n) -> p k n", n=M)
                    j1 = fold_pool.tile([128, KB * (M // 2)], f16, tag="j1")
                    v1 = j1[:, :].rearrange("p (k n) -> p k n", n=M // 2)
                    nc.vector.tensor_tensor(
                        out=v1, in0=c3[:, :, 0:M // 2], in1=c3[:, :, M // 2:M],
                        op=mybir.AluOpType.min)
                    j2 = fold_pool.tile([128, KB * (M // 4)], f16, tag="j2")
                    v2 = j2[:, :].rearrange("p (k n) -> p k n", n=M // 4)
                    nc.vector.tensor_tensor(
                        out=v2, in0=v1[:, :, 0:M // 4], in1=v1[:, :, M // 4:M // 2],
                        op=mybir.AluOpType.min)
                    j3 = fold_pool.tile([128, KB * (M // 8)], f16, tag="j3")
                    v3 = j3[:, :].rearrange("p (k n) -> p k n", n=M // 8)
                    nc.vector.tensor_tensor(
                        out=v3, in0=v2[:, :, 0:M // 8], in1=v2[:, :, M // 8:M // 4],
                        op=mybir.AluOpType.min)
                    j4 = fold_pool.tile([128, KB * (M // 16)], f16, tag="j4")
                    v4 = j4[:, :].rearrange("p (k n) -> p k n", n=M // 16)
                    nc.vector.tensor_tensor(
                        out=v4, in0=v3[:, :, 0:M // 16], in1=v3[:, :, M // 16:M // 8],
                        op=mybir.AluOpType.min)
                    nc.vector.tensor_reduce(
                        out=BMINS[:, b0:b0 + KB], in_=v4,
                        axis=mybir.AxisListType.X, op=mybir.AluOpType.min,
                    )
                    if b0 + KB == n_act:
                        # all ACT-route + early DVE results ready: ship early
                        nc.sync.dma_start(out=res[:, 0:n_act], in_=BMINS[:, :])
                        nc.sync.dma_start(
                            out=res[:, n_act:n_act + 2 * N_EARLY_DVE],
                            in_=DMINS[:, 0:2 * N_EARLY_DVE])

            nc.sync.dma_start(out=res[:, n_act + 2 * N_EARLY_DVE:],
                              in_=DMINS[:, 2 * N_EARLY_DVE:])

    import json as _json

    pruned = _prune_redundant_waits(_json.loads(nc.to_json_bytes()))
    blob = _json.dumps(pruned).encode()
    nc.to_json_bytes = lambda: blob  # instance override read by bass2jax
    return nc


def _host_prep(recon_points: np.ndarray, gt_points: np.ndarray):
    """Build per-core [KEXT, AB_COLS] fused operands (reordered cols)."""
    in_maps = []
    for core in range(N_CORES):
        b, h = divmod(core, 2)
        ii = _II[h * HALF:(h + 1) * HALF]
        jj = _JJ[h * HALF:(h + 1) * HALF]
        rec = recon_points[b].astype(np.float32)          # [64, 3]
        start, end = rec[ii], rec[jj]                     # [1008, 3]

        A = np.zeros((5, RPAD), dtype=np.float32)
        for fi, f in enumerate(FRACS):
            k = (start * np.float32(f) + end * np.float32(1.0 - f)).astype(np.float32)
            cols = slice(fi * HALF, (fi + 1) * HALF)
            A[0:3, cols] = k.T
            A[3, cols] = (k.astype(np.float64) ** 2).sum(1).astype(np.float32)
            A[4, cols] = 1.0
        ep_pts = rec[h * EPC:(h + 1) * EPC]               # [32, 3]
        ep = slice(NF * HALF, NF * HALF + EPC)
        A[0:3, ep] = ep_pts.T
        A[3, ep] = (ep_pts.astype(np.float64) ** 2).sum(1).astype(np.float32)
        A[4, ep] = 1.0

        g = gt_points[b].astype(np.float32)               # [2048, 3]
        Bm = np.empty((5, M), dtype=np.float32)
        Bm[0:3] = np.float32(-2.0) * g.T
        Bm[3] = 1.0
        Bm[4] = (g.astype(np.float64) ** 2).sum(1).astype(np.float32)

        Ah, Al, Ar = _split3_bf16(A)
        Bh, Bl, Br = _split3_bf16(Bm)
        # Product groups, largest magnitude first: hh | hl lh | hr rh ll | lr rl
        A_ext = np.concatenate([Ah, Ah, Al, Ah, Ar, Al, Al, Ar], axis=0)  # [40, RPAD]
        B_ext = np.concatenate([Bh, Bl, Bh, Br, Bh, Bl, Br, Bl], axis=0)  # [40, M]
        # column order: [PF tile0 | GT | PF tiles 1..23]
        ab = np.concatenate([A_ext[:, 0:128], B_ext, A_ext[:, 128:]], axis=1)
        in_maps.append({"ab": np.ascontiguousarray(ab)})
    return in_maps


def _host_assemble(results) -> np.ndarray:
    n_dve = len(DVE_SET)
    n_act = NTILES - n_dve
    out = np.empty((B, P), dtype=np.float32)
    E_full = np.empty((B, NPTS), dtype=np.float32)
    mins_by_core = []
    for core in range(N_CORES):
        b, h = divmod(core, 2)
        res = np.asarray(results[core]["res"], dtype=np.float32)
        mins = np.empty((NTILES, 128), dtype=np.float32)
        d_idx = a_idx = 0
        for t in range(NTILES):
            if t in DVE_SET:
                mins[t] = np.minimum(res[:, n_act + 2 * d_idx],
                                     res[:, n_act + 2 * d_idx + 1])
                d_idx += 1
            else:
                mins[t] = res[:, a_idx]
                a_idx += 1
        flat = mins.reshape(-1)                 # row r = t*128 + partition
        mins_by_core.append(flat)
        E_full[b, h * EPC:(h + 1) * EPC] = flat[NF * HALF:NF * HALF + EPC]
    for core in range(N_CORES):
        b, h = divmod(core, 2)
        ii = _II[h * HALF:(h + 1) * HALF]
        jj = _JJ[h * HALF:(h + 1) * HALF]
        flat = mins_by_core[core]
        s3 = flat[:NF * HALF].reshape(NF, HALF).sum(axis=0)
        out[b, h * HALF:(h + 1) * HALF] = (
            s3 + E_full[b, ii] + E_full[b, jj]) * np.float32(0.2)
    return out


_NC_CACHE = None


def _get_nc() -> bass.Bass:
    global _NC_CACHE
    if _NC_CACHE is None:
        _NC_CACHE = _build_nc()
    return _NC_CACHE


def run(recon_points: np.ndarray, gt_points: np.ndarray, **spmd_kwargs):
    """Run on 8 NeuronCores; returns (output [4, 2016], BassKernelResults)."""
    nc = _get_nc()
    in_maps = _host_prep(recon_points, gt_points)
    r = run_bass_kernel_spmd(nc, in_maps, list(range(N_CORES)), **spmd_kwargs)
    return _host_assemble(r.results), r


def kernel(recon_points: np.ndarray, gt_points: np.ndarray) -> np.ndarray:
    recon_points = np.asarray(recon_points, dtype=np.float32)
    gt_points = np.asarray(gt_points, dtype=np.float32)
    out, _ = run(recon_points, gt_points)
    return out


# revision 5
# speedup vs baseline: 1.0108x; 1.0108x over previous
"""Trainium2 Bass kernel for nn_ComputeEdgeLoss — v2.

Computes, for each batch b and lower-triangular pair (i, j) of the 64
recon keypoints, the mean over 5 interpolated segment points of the min
squared distance to the 2048 gt points of that batch.

Strategy (v2 changes over the 79 us baseline)
---------------------------------------------
* 24 row-tiles instead of 25: the 64 endpoint rows are split 32/32
  between the two cores of each batch pair (host assembly re-merges),
  so each core computes 3*1008 + 32 = 3056 rows in 24 128-row tiles.
  PE work drops 4%.
* Input columns reordered [PF_t0 | GT | PF_rest] and split into 5 DMAs
  so the first matmul's dependencies (PF tile 0 + GT chunks 0-1) arrive
  in one small early DMA instead of after the full 420 KB load.
* Tail: the last two tiles are DVE-direct and the final ACT fold batch
  lands at tile 21, so only one ~1.5 us reduce trails the final matmul
  vs ~7 us of serial DVE work before; ACT-route results ship in an
  early DMA so the final transfer is 12 columns.

Math: for an interp point k and gt point g,
    ||k - g||^2 = a . b,  a = [kx, ky, kz, ||k||^2, 1],
                          b = [-2gx, -2gy, -2gz, 1, ||g||^2]
fp32 inputs split into three bf16 terms (27-bit fidelity) and the six
>=2^-24 product groups folded into 40 bf16 contraction rows; one PE
matmul emits a [128 x 512] fp32 distance block per 512 cycles.
"""

import numpy as np

import concourse.bass as bass
import concourse.mybir as mybir
import concourse.tile as tile
from concourse.bass_utils import run_bass_kernel_spmd

# Problem shape (hardcoded per contest rules).
B = 4          # batches
NPTS = 64      # recon points per batch
M = 2048       # gt points per batch
P = NPTS * (NPTS - 1) // 2   # 2016 pairs
HALF = P // 2                # 1008 pairs per core
N_CORES = 8
FRACS = (0.25, 0.5, 0.75)    # interior interpolation fractions
NF = len(FRACS)
EPC = NPTS // 2              # endpoints per core (32)
ROWS = NF * HALF + EPC       # 3056 real rows per core
NTILES = 24                  # ceil(3056 / 128)
RPAD = NTILES * 128          # 3072 padded rows
KEXT = 40                    # split contraction depth (8 groups x 5)
GT_CHUNK = 512               # PSUM bank free size (fp32)
HM = M // 2                  # half-tile free size [128, 1024]

# AB column layout: [PF tile0 (128) | GT (2048) | PF tiles 1..23 (2944)]
AB_COLS = RPAD + M
GT0 = 128                    # GT column offset

_II, _JJ = np.tril_indices(NPTS, -1)   # pair order matches reference

# Tiles drained by DVE directly from PSUM (fp32 tensor_reduce per half).
# The rest: ACT fp16-copy to SBUF, then a batched DVE fold tree (fp16
# tensor_tensor at 2 elem/cycle) per KB tiles.  The last fold batch ends
# at tile 19 so its ~4 us chain overlaps tiles 20-23's matmuls, and the
# final four tiles are DVE-direct so only ~2 half-reduces trail the last
# matmul (a trailing fold batch cost ~5.8 us of serial DVE before).
# (Fused tensor_tensor_reduce and custom DVE ops both fail this walrus
# build's codegen with "ISA wrong length"; GpSimd has no TensorTensor.)
DVE_SET = (2, 6, 10, 14, 22, 23)
KB = 3                       # ACT tiles per batched fold chain
N_EARLY_DVE = 4              # leading DVE_SET tiles shipped in the early DMA


def _split3_bf16(x: np.ndarray):
    """Split fp32 x into three bf16 terms with x ~= h + l + r (27-bit
    significand fidelity; differences are Sterbenz-exact in fp32)."""
    import ml_dtypes

    bf16 = ml_dtypes.bfloat16
    x = np.ascontiguousarray(x, dtype=np.float32)
    h = x.astype(bf16)
    l32 = (x - h.astype(np.float32)).astype(np.float32)
    l = l32.astype(bf16)
    r = (l32 - l.astype(np.float32)).astype(np.float32).astype(bf16)
    return h, l, r


_COMPUTE_ENGINES = {"PE", "DVE", "Activation", "Pool"}


def _register_min_reduce_op():
    """Register a custom DVE op: out = min(in0, in1) elementwise, with a
    fused min-accumulate over the free dim into accum_out (init = s0).

    Stock tensor_reduce runs at 1 element/cycle on this HW, and walrus
    rejects the stock InstTensorTensorReduce ("ISA wrong length"), but a
    custom two-source DVE body consumes 2 elements/cycle: reading one
    half of each distance tile from PSUM (in0) and the ACT-staged fp16
    other half from SBUF (in1) drains a whole [128, 2048] tile in one
    ~1.3 us op."""
    import concourse.dve_ops as dops
    from concourse.dve_spec import C0, Spec, Src0, Src1, lower, minn
    from concourse.dve_uop import DveOpSpec

    name = "ANT_TT_MIN_REDUCE_EDGE"
    for o in dops.OPS:
        if o.name == name:
            return o

    def _ref(in0, in1, c0, c1, c2):
        return np.minimum(in0.astype(np.float32), in1.astype(np.float32))

    spec = Spec(body=minn(Src0, Src1), accum=minn, accum_init=C0, reference=_ref)
    row = max(dops._SUB_OPCODE_FOR_NAME.values()) + 1
    assert row < 0x20
    ver = "v3"  # TRN2
    sha = DveOpSpec(
        name=name, opcode=row, uops=lower(spec, ver=ver), rd1_en=True
    ).sha(ver)
    op = dops.DveOp(name, spec, subdim=False, uops_sha={ver: sha})
    dops.OPS.append(op)
    dops.CUSTOM_DVE_SPECS[name] = spec
    dops._SUB_OPCODE_FOR_NAME[name] = row
    return op


def _prune_redundant_waits(bir: dict) -> dict:
    """Reduce every instruction to at most ONE sync-wait.

    This walrus build accepts only one sync-wait per instruction, but
    Tile's semaphore pass is not transitively minimal.  We reconstruct
    per-instruction guaranteed semaphore lower bounds (vector clocks
    over the scheduled program order) and delete implied waits; any
    residual multi-wait instruction is split into single-wait Drain
    carriers on the same engine.

    Soundness model: per-engine in-order dispatch; in-order completion
    for compute engines; per-semaphore in-order completion for DMA-queue
    sems (each DMAHW sem belongs to one queue).  Only monotone
    (inc-only) semaphores with sem-ge-imm waits participate.
    """
    fn = bir["functions"][0]

    contrib_engines: dict[int, set] = {}
    monotone: dict[int, bool] = {}
    for b in fn["blocks"]:
        for ins in b["instructions"]:
            sy = ins.get("sync_info") or {}
            for u in sy.get("on_update") or []:
                if u.get("sync_type") != "semaphore":
                    continue
                s = u["id"]
                contrib_engines.setdefault(s, set()).add(ins.get("engine"))
                ok = u.get("update_mode") == "sem-inc"
                monotone[s] = monotone.get(s, True) and ok

    def usable(s):
        return monotone.get(s, False)

    def mergemax(dst, src):
        for k, v in src.items():
            if dst.get(k, -1) < v:
                dst[k] = v

    prev_start_know: dict[str, dict] = {}
    cum: dict[int, int] = {}            # sem -> cumulative inc in walk order
    comp_know: list[dict] = []          # per walk index
    sem_reach: dict[int, list] = {}     # sem -> [(value_after, walk_idx)]
    dropped = 0
    walk_idx = 0

    for b in fn["blocks"]:
        new_insts = []
        for ins in b["instructions"]:
            eng = ins.get("engine")
            sy = ins.get("sync_info") or {}
            waits = list(sy.get("on_wait") or [])

            def know_from(wlist):
                know = dict(prev_start_know.get(eng, {}))
                for w in wlist:
                    if (w.get("sync_type") != "semaphore"
                            or w.get("wait_mode") != "sem-ge-imm"):
                        continue
                    s, v = w["id"], w["wait_value"]
                    if not usable(s):
                        continue
                    if know.get(s, -1) < v:
                        know[s] = v
                    if len(contrib_engines.get(s, ())) == 1:
                        for after, pidx in sem_reach.get(s, ()):
                            if after >= v:
                                mergemax(know, comp_know[pidx])
                                break
                return know

            if len(waits) > 1:
                kept = list(waits)
                changed = True
                while changed and len(kept) > 1:
                    changed = False
                    for w in list(kept):
                        others = [x for x in kept if x is not w]
                        if (w.get("sync_type") == "semaphore"
                                and w.get("wait_mode") == "sem-ge-imm"
                                and usable(w["id"])
                                and know_from(others).get(w["id"], -1)
                                >= w["wait_value"]):
                            kept.remove(w)
                            dropped += 1
                            changed = True
                            break
                if len(kept) > 1:
                    for k, w in enumerate(kept[:-1]):
                        new_insts.append({
                            "name": f"{ins['name']}-w{k}",
                            "engine": eng, "ins": [], "outs": [],
                            "opcode": "Drain",
                            "sync_info": {"on_wait": [w], "on_update": []},
                        })
                        walk_idx += 1
                        comp_know.append(dict(prev_start_know.get(eng, {})))
                    kept = kept[-1:]
                if len(kept) != len(waits):
                    if not sy:
                        ins["sync_info"] = sy = {"on_update": []}
                    sy["on_wait"] = kept
                    waits = kept

            start_know = know_from(waits)
            prev_start_know[eng] = start_know

            own = set()
            for u in sy.get("on_update") or []:
                if (u.get("sync_type") == "semaphore"
                        and u.get("update_mode") == "sem-inc"):
                    s = u["id"]
                    cum[s] = cum.get(s, 0) + u.get("update_value", 1)
                    sem_reach.setdefault(s, []).append((cum[s], walk_idx))
                    own.add(s)
            ck = dict(start_know)
            for s in own:
                if usable(s) and len(contrib_engines.get(s, ())) == 1:
                    if ck.get(s, -1) < cum[s]:
                        ck[s] = cum[s]
            if eng in _COMPUTE_ENGINES:
                for s, c in cum.items():
                    if (usable(s) and contrib_engines.get(s) == {eng}
                            and ck.get(s, -1) < c):
                        ck[s] = c
            comp_know.append(ck)
            new_insts.append(ins)
            walk_idx += 1
        b["instructions"] = new_insts
    return bir


def _build_nc() -> bass.Bass:
    nc = bass.Bass()
    ab = nc.declare_dram_parameter("ab", [KEXT, AB_COLS], mybir.dt.bfloat16,
                                   isOutput=False)
    n_dve = len(DVE_SET)
    n_act = NTILES - n_dve
    assert n_act % KB == 0
    # res columns: [0:n_act] fp32 B-mins by act-ordinal; then 2 per DVE tile
    res = nc.declare_dram_parameter("res", [128, n_act + 2 * n_dve],
                                    mybir.dt.float32, isOutput=True)

    f32 = mybir.dt.float32
    f16 = mybir.dt.float16

    with tile.TileContext(nc) as tc:
        with (
            tc.tile_pool(name="const", bufs=1) as const_pool,
            tc.tile_pool(name="psum", bufs=4, space="PSUM") as psum_pool,
            tc.tile_pool(name="cp", bufs=2) as cp_pool,
            tc.tile_pool(name="fold", bufs=2) as fold_pool,
        ):
            AB = const_pool.tile([KEXT, AB_COLS], mybir.dt.bfloat16, name="AB")
            DMINS = const_pool.tile([128, 2 * n_dve], f32, name="DMINS")
            BMINS = const_pool.tile([128, n_act], f32, name="BMINS")

            # 5 input DMAs ordered by first consumer:
            #   D0 = PF_t0 + GT chunks 0-1, D1 = GT chunks 2-3,
            #   D2 = PF tiles 1-4, D3/D4 = the rest.
            cuts = (0, GT0 + 2 * GT_CHUNK, GT0 + M, GT0 + M + 4 * 128,
                    GT0 + M + 1664, AB_COLS)
            for i in range(len(cuts) - 1):
                nc.sync.dma_start(out=AB[:, cuts[i]:cuts[i + 1]],
                                  in_=ab[:, cuts[i]:cuts[i + 1]])
            GT = AB[:, GT0:GT0 + M]

            def pf_tile(t):
                if t == 0:
                    return AB[:, 0:128]
                return AB[:, GT0 + M + (t - 1) * 128:GT0 + M + t * 128]

            d_idx = 0
            a_idx = 0
            cp_cur = None
            for t in range(NTILES):
                lhsT = pf_tile(t)
                halves = []
                for hh in range(2):
                    ptile = psum_pool.tile([128, HM], f32, tag="ptile")
                    halves.append(ptile)
                    for c in range(2):
                        sl_g = slice((2 * hh + c) * GT_CHUNK,
                                     (2 * hh + c + 1) * GT_CHUNK)
                        sl_p = slice(c * GT_CHUNK, (c + 1) * GT_CHUNK)
                        nc.tensor.matmul(
                            out=ptile[:, sl_p], lhsT=lhsT, rhs=GT[:, sl_g],
                            start=True, stop=True,
                        )
                if t in DVE_SET:
                    for hh in range(2):
                        nc.vector.tensor_reduce(
                            out=DMINS[:, 2 * d_idx + hh:2 * d_idx + hh + 1],
                            in_=halves[hh][:, :],
                            axis=mybir.AxisListType.X, op=mybir.AluOpType.min,
                        )
                    d_idx += 1
                    continue

                j = a_idx % KB
                if j == 0:
                    cp_cur = cp_pool.tile([128, KB * M], f16, tag="cp")
                for hh in range(2):
                    nc.scalar.copy(
                        cp_cur[:, j * M + hh * HM:j * M + (hh + 1) * HM],
                        halves[hh][:, :])
                a_idx += 1
                if j == KB - 1:
                    b0 = a_idx - KB
                    c3 = cp_cur[:, :].rearrange("p (k n) -> p k n", n=M)
                    j1 = fold_pool.tile([128, KB * (M // 2)], f16, tag="j1")
                    v1 = j1[:, :].rearrange("p (k n) -> p k n", n=M // 2)
                    nc.vector.tensor_tensor(
                        out=v1, in0=c3[:, :, 0:M // 2], in1=c3[:, :, M // 2:M],
                        op=mybir.AluOpType.min)
                    j2 = fold_pool.tile([128, KB * (M // 4)], f16, tag="j2")
                    v2 = j2[:, :].rearrange("p (k n) -> p k n", n=M // 4)
                    nc.vector.tensor_tensor(
                        out=v2, in0=v1[:, :, 0:M // 4], in1=v1[:, :, M // 4:M // 2],
                        op=mybir.AluOpType.min)
                    j3 = fold_pool.tile([128, KB * (M // 8)], f16, tag="j3")
                    v3 = j3[:, :].rearrange("p (k n) -> p k n", n=M // 8)
                    nc.vector.tensor_tensor(
                        out=v3, in0=v2[:, :, 0:M // 8], in1=v2[:, :, M // 8:M // 4],
                        op=mybir.AluOpType.min)
                    j4 = fold_pool.tile([128, KB * (M // 16)], f16, tag="j4")
                    v4 = j4[:, :].rearrange("p (k n) -> p k n", n=M // 16)
                    nc.vector.tensor_tensor(
                        out=v4, in0=v3[:, :, 0:M // 16], in1=v3[:, :, M // 16:M // 8],
                        op=mybir.AluOpType.min)
                    nc.vector.tensor_reduce(
                        out=BMINS[:, b0:b0 + KB], in_=v4,
                        axis=mybir.AxisListType.X, op=mybir.AluOpType.min,
                    )
                    if b0 + KB == n_act:
                        # all ACT-route + early DVE results ready: ship early
                        nc.sync.dma_start(out=res[:, 0:n_act], in_=BMINS[:, :])
                        nc.sync.dma_start(
                            out=res[:, n_act:n_act + 2 * N_EARLY_DVE],
                            in_=DMINS[:, 0:2 * N_EARLY_DVE])

            nc.sync.dma_start(out=res[:, n_act + 2 * N_EARLY_DVE:],
                              in_=DMINS[:, 2 * N_EARLY_DVE:])

    import json as _json

    pruned = _prune_redundant_waits(_json.loads(nc.to_json_bytes()))
    blob = _json.dumps(pruned).encode()
    nc.to_json_bytes = lambda: blob  # instance override read by bass2jax
    return nc


def _host_prep(recon_points: np.ndarray, gt_points: np.ndarray):
    """Build per-core [KEXT, AB_COLS] fused operands (reordered cols)."""
    in_maps = []
    for core in range(N_CORES):
        b, h = divmod(core, 2)
        ii = _II[h * HALF:(h + 1) * HALF]
        jj = _JJ[h * HALF:(h + 1) * HALF]
        rec = recon_points[b].astype(np.float32)          # [64, 3]
        start, end = rec[ii], rec[jj]                     # [1008, 3]

        A = np.zeros((5, RPAD), dtype=np.float32)
        for fi, f in enumerate(FRACS):
            k = (start * np.float32(f) + end * np.float32(1.0 - f)).astype(np.float32)
            cols = slice(fi * HALF, (fi + 1) * HALF)
            A[0:3, cols] = k.T
            A[3, cols] = (k.astype(np.float64) ** 2).sum(1).astype(np.float32)
            A[4, cols] = 1.0
        ep_pts = rec[h * EPC:(h + 1) * EPC]               # [32, 3]
        ep = slice(NF * HALF, NF * HALF + EPC)
        A[0:3, ep] = ep_pts.T
        A[3, ep] = (ep_pts.astype(np.float64) ** 2).sum(1).astype(np.float32)
        A[4, ep] = 1.0

        g = gt_points[b].astype(np.float32)               # [2048, 3]
        Bm = np.empty((5, M), dtype=np.float32)
        Bm[0:3] = np.float32(-2.0) * g.T
        Bm[3] = 1.0
        Bm[4] = (g.astype(np.float64) ** 2).sum(1).astype(np.float32)

        Ah, Al, Ar = _split3_bf16(A)
        Bh, Bl, Br = _split3_bf16(Bm)
        # Product groups, largest magnitude first: hh | hl lh | hr rh ll | lr rl
        A_ext = np.concatenate([Ah, Ah, Al, Ah, Ar, Al, Al, Ar], axis=0)  # [40, RPAD]
        B_ext = np.concatenate([Bh, Bl, Bh, Br, Bh, Bl, Br, Bl], axis=0)  # [40, M]
        # column order: [PF tile0 | GT | PF tiles 1..23]
        ab = np.concatenate([A_ext[:, 0:128], B_ext, A_ext[:, 128:]], axis=1)
        in_maps.append({"ab": np.ascontiguousarray(ab)})
    return in_maps


def _host_assemble(results) -> np.ndarray:
    n_dve = len(DVE_SET)
    n_act = NTILES - n_dve
    out = np.empty((B, P), dtype=np.float32)
    E_full = np.empty((B, NPTS), dtype=np.float32)
    mins_by_core = []
    for core in range(N_CORES):
        b, h = divmod(core, 2)
        res = np.asarray(results[core]["res"], dtype=np.float32)
        mins = np.empty((NTILES, 128), dtype=np.float32)
        d_idx = a_idx = 0
        for t in range(NTILES):
            if t in DVE_SET:
                mins[t] = np.minimum(res[:, n_act + 2 * d_idx],
                                     res[:, n_act + 2 * d_idx + 1])
                d_idx += 1
            else:
                mins[t] = res[:, a_idx]
                a_idx += 1
        flat = mins.reshape(-1)                 # row r = t*128 + partition
        mins_by_core.append(flat)
        E_full[b, h * EPC:(h + 1) * EPC] = flat[NF * HALF:NF * HALF + EPC]
    for core in range(N_CORES):
        b, h = divmod(core, 2)
        ii = _II[h * HALF:(h + 1) * HALF]
        jj = _JJ[h * HALF:(h + 1) * HALF]
        flat = mins_by_core[core]
        s3 = flat[:NF * HALF].reshape(NF, HALF).sum(axis=0)
        out[b, h * HALF:(h + 1) * HALF] = (
            s3 + E_full[b, ii] + E_full[b, jj]) * np.float32(0.2)
    return out


_NC_CACHE = None


def _get_nc() -> bass.Bass:
    global _NC_CACHE
    if _NC_CACHE is None:
        _NC_CACHE = _build_nc()
    return _NC_CACHE


def run(recon_points: np.ndarray, gt_points: np.ndarray, **spmd_kwargs):
    """Run on 8 NeuronCores; returns (output [4, 2016], BassKernelResults)."""
    nc = _get_nc()
    in_maps = _host_prep(recon_points, gt_points)
    r = run_bass_kernel_spmd(nc, in_maps, list(range(N_CORES)), **spmd_kwargs)
    return _host_assemble(r.results), r


def kernel(recon_points: np.ndarray, gt_points: np.ndarray) -> np.ndarray:
    recon_points = np.asarray(recon_points, dtype=np.float32)
    gt_points = np.asarray(gt_points, dtype=np.float32)
    out, _ = run(recon_points, gt_points)
    return out
